# revision 25
# baseline (speedup 1.0000x reference)
"""Trainium2 Bass kernel for nn_Network_81862076662591 (sampling network).

Self-contained: takes FULL inputs (as produced by the problem's
setup_inputs), data-parallel shards batch B=256 over 8 NeuronCores
(32 rows each, per-iteration weights replicated), runs the fused
LSTM + gumbel-argmax sampling + MLP scan on-device, and returns the
full [256, 10, 100] output.

Numerical scheme: every weight-stationary matmul runs as a 3-pass
bf16 hi/lo decomposition (A_hi@W_hi + A_lo@W_hi + A_hi@W_lo), which
is exact to ~2^-18 — verified offline to reproduce the fp32 argmax
trajectory exactly (min top-2 gumbel gap is 6.1e-5, pert error
~1.7e-6).  The streamed Wg2 matmul stays true fp32 (2 cyc/row).
Weights are host-packed into per-iteration partition-major blobs so
each weight DMA is 128 contiguous multi-KB lines (near line-rate).
Dummy matmuls are threaded through the sampling gap to keep the PE
HAM clock at full rate.
"""
from contextlib import ExitStack

import numpy as np
import ml_dtypes

import concourse.bass as bass
import concourse.mybir as mybir
import concourse.tile as tile
from concourse.vector_clock import ScopedClock
from concourse.bass_utils import run_bass_kernel_spmd

F32 = mybir.dt.float32
BF16 = mybir.dt.bfloat16
F16 = mybir.dt.float16
U32 = mybir.dt.uint32
ALU = mybir.AluOpType
ACTF = mybir.ActivationFunctionType

NCORES = 8
B = 32          # per-core batch
D = 784
G = 256
H = 128
T = 100
NEGBIG = -1.0e9
NK = 7                       # D chunks: 6 x 128 + 1 x 16
CH = [128] * 6 + [16]
KOF = [128 * k for k in range(7)]
# blobA free-dim offsets (per k-chunk, in bf16 elems)
A_W1HI, A_W1LO, A_W2HI, A_W2LO, A_LINE = 0, 784, 1568, 1824, 2080
# blobB free-dim offsets (bf16 elems)
B_WG2, B_WG1HI, B_WG1LO, B_W3HI, B_W3LO, B_W4HI, B_W4LO, B_WF2, B_LINE = (
    0, 3136, 3392, 3648, 3904, 4160, 4288, 4416, 4426)


class _TileContextSplitDrain(tile.TileContext):
    """This walrus build rejects >1 sem-wait on the kernel-tail Drain;
    split the accumulated waits across several sequential drains."""

    def _drain_and_barrier(self, tick_clock, wait_clock):
        drain_inst = self.nc.sync.drain()
        wait_clock.add_sem_waits(
            drain_inst.ins, ScopedClock({None: tick_clock.global_clock}))
        si = drain_inst.ins.sync_info
        waits = list(si.on_wait or []) if si is not None else []
        if len(waits) > 1:
            si.on_wait = [waits[0]]
            for w in waits[1:]:
                d2 = self.nc.sync.drain()
                if d2.ins.sync_info is None:
                    d2.ins.sync_info = mybir.SyncInfo(on_wait=[w], on_update=[])
                else:
                    d2.ins.sync_info.on_wait = [w]
        self.nc.all_engine_barrier()
        assert self.sems is not None
        popped = self.nc._tile_sem_poison_stack.pop()
        assert popped is self._sem_poison
        self.nc.clear_and_free_semaphores(list(self.sems.allocated().values()))
        self.nc.all_engine_barrier()


def _split_multi_waits(nc, limit=1):
    """This walrus accepts only `limit` sem-waits per instruction; move the
    excess onto same-engine sequencer NOPs inserted immediately before."""
    import copy

    proto = nc.vector.isa(nc.isa.Opcode.NEURON_ISA_TPB_OPCODE_NOP, {}).ins
    nop_ctr = [0]

    def make_nop(engine, waits):
        nop = copy.deepcopy(proto)
        nop_ctr[0] += 1
        nop.name = f"waitnop-{nop_ctr[0]}"
        nop.engine = engine
        nop.sync_info = mybir.SyncInfo(on_wait=list(waits), on_update=[])
        return nop

    skip = ("InstAllEngineBarrier", "InstEventSemaphore")
    for fn in nc.m.functions:
        for bb in fn.blocks:
            insts = bb.instructions
            if insts and insts[-1] is proto:
                insts.pop()
            out = []
            for inst in insts:
                si = inst.sync_info
                waits = list(si.on_wait or []) if si is not None else []
                if len(waits) > limit and type(inst).__name__ not in skip:
                    for i in range(0, len(waits) - limit, limit):
                        out.append(make_nop(inst.engine, waits[i:i + limit]))
                    si.on_wait = waits[len(waits) - limit:]
                out.append(inst)
            bb.instructions[:] = out


def _declare_params(nc):
    dp = nc.declare_dram_parameter
    p = {}
    p["blobA"] = dp("blobA", [T, 128, 6 * A_LINE], BF16, isOutput=False)
    p["blobA2"] = dp("blobA2", [T, 16, A_LINE], BF16, isOutput=False)
    p["blobB"] = dp("blobB", [T, 128, B_LINE], BF16, isOutput=False)
    p["gum"] = dp("gum", [T, B, D], F32, isOutput=False)
    p["wihhi"] = dp("wihhi", [H, 4 * H], BF16, isOutput=False)
    p["wihlo"] = dp("wihlo", [H, 4 * H], BF16, isOutput=False)
    p["whhhi"] = dp("whhhi", [H, 4 * H], BF16, isOutput=False)
    p["whhlo"] = dp("whhlo", [H, 4 * H], BF16, isOutput=False)
    p["xhi"] = dp("xhi", [128, NK, B], BF16, isOutput=False)
    p["xlo"] = dp("xlo", [128, NK, B], BF16, isOutput=False)
    p["iota32"] = dp("iota32", [B, D], F32, isOutput=False)
    p["identbf"] = dp("identbf", [B, B], BF16, isOutput=False)
    p["out"] = dp("out", [B, 10 * T], F32, isOutput=True)
    return p


def _add_dep(after_ins, before_ins, reason):
    from bass_rust import add_dep_helper
    add_dep_helper(after_ins, before_ins, reason=reason)


def _mm3(nc, out, whi, wlo, ahi, alo, first, last):
    """3-pass hi/lo matmul accumulate: out += ahi'whi + alo'whi + ahi'wlo."""
    nc.tensor.matmul(out, whi, ahi, start=first, stop=False)
    nc.tensor.matmul(out, whi, alo, start=False, stop=False)
    nc.tensor.matmul(out, wlo, ahi, start=False, stop=last)


def _build(ctx, tc, p, w_bufs=2):
    nc = tc.nc

    const_pool = ctx.enter_context(tc.tile_pool(name="const", bufs=1))
    state_pool = ctx.enter_context(tc.tile_pool(name="state", bufs=1))
    wpool = ctx.enter_context(tc.tile_pool(name="w", bufs=w_bufs))
    psum = ctx.enter_context(tc.tile_pool(name="ps", bufs=1, space="PSUM"))

    WIHHI = const_pool.tile([H, 4 * H], BF16, tag="wihhi")
    WIHLO = const_pool.tile([H, 4 * H], BF16, tag="wihlo")
    WHHHI = const_pool.tile([H, 4 * H], BF16, tag="whhhi")
    WHHLO = const_pool.tile([H, 4 * H], BF16, tag="whhlo")
    XHI = const_pool.tile([128, NK, B], BF16, tag="xhi")
    XLO = const_pool.tile([128, NK, B], BF16, tag="xlo")
    IOTA32 = const_pool.tile([B, D], F32, tag="iota32")
    IDENTBF = const_pool.tile([B, B], BF16, tag="identbf")
    DUMC = const_pool.tile([128, 512], F32, tag="dumc")   # dummy-MM fodder
    nc.vector.memset(DUMC[:], 0.0)
    for name, tl in (("wihhi", WIHHI), ("wihlo", WIHLO), ("whhhi", WHHHI),
                     ("whhlo", WHHLO), ("xhi", XHI), ("xlo", XLO),
                     ("iota32", IOTA32), ("identbf", IDENTBF)):
        nc.sync.dma_start(tl[:], p[name].ap())

    # state
    AHI = state_pool.tile([H, B], BF16, tag="ahi")    # lin hi/lo
    ALO = state_pool.tile([H, B], BF16, tag="alo")
    HHI = state_pool.tile([H, B], BF16, tag="hhi")
    HLO = state_pool.tile([H, B], BF16, tag="hlo")
    H32 = state_pool.tile([H, B], F32, tag="h32")
    C32 = state_pool.tile([H, B], F32, tag="c32")
    SIG3 = state_pool.tile([H, 3, B], F32, tag="sig3")
    TG = state_pool.tile([H, B], F32, tag="tg")
    TC_ = state_pool.tile([H, B], F32, tag="tc")
    U0 = state_pool.tile([H, B], F32, tag="u0")
    U1 = state_pool.tile([H, B], F32, tag="u1")
    G1F = state_pool.tile([H, 2, B], F32, tag="g1f")
    MEMBF = state_pool.tile([128, NK, B], BF16, tag="membf")
    YTHI = state_pool.tile([128, NK, B], BF16, tag="ythi")
    YTLO = state_pool.tile([128, NK, B], BF16, tag="ytlo")
    A1F = state_pool.tile([128, NK, B], F32, tag="a1f")
    A1HI = state_pool.tile([128, NK, B], BF16, tag="a1hi")
    A1LO = state_pool.tile([128, NK, B], BF16, tag="a1lo")
    A2F = state_pool.tile([H, 2, B], F32, tag="a2f")
    A2HI = state_pool.tile([H, 2, B], BF16, tag="a2hi")
    A2LO = state_pool.tile([H, 2, B], BF16, tag="a2lo")
    A3F = state_pool.tile([H, B], F32, tag="a3f")
    A3HI = state_pool.tile([H, B], BF16, tag="a3hi")
    A3LO = state_pool.tile([H, B], BF16, tag="a3lo")
    A4F = state_pool.tile([H, B], F32, tag="a4f")
    NEGMEM = state_pool.tile([B, D], F32, tag="negmem")
    GMEM = state_pool.tile([B, D], F32, tag="gmem")
    PERT = state_pool.tile([B, D], F32, tag="pert")
    MAX2 = state_pool.tile([B, 2, 8], F32, tag="max2")
    MAX8 = state_pool.tile([B, 8], F32, tag="max8")
    IDX8 = state_pool.tile([B, 8], U32, tag="idx8")
    IDXF = state_pool.tile([B, 1], F32, tag="idxf")
    HARDBF = state_pool.tile([B, D], BF16, tag="hardbf")
    NEGHARD = state_pool.tile([B, D], F32, tag="neghard")
    SOUT = state_pool.tile([B, 10, T], F32, tag="sout")

    nc.vector.memset(AHI[:], 0.0)
    nc.vector.memset(ALO[:], 0.0)
    nc.vector.memset(HHI[:], 0.0)
    nc.vector.memset(HLO[:], 0.0)
    nc.vector.memset(C32[:], 0.0)
    nc.vector.memset(MEMBF[:], 0.0)
    nc.vector.memset(NEGMEM[:], 0.0)

    for t in range(T):
        BA = wpool.tile([128, 6, A_LINE], BF16, tag="ba")
        BA2 = wpool.tile([16, A_LINE], BF16, tag="ba2")
        BB = wpool.tile([128, B_LINE], BF16, tag="bb")
        GUM = wpool.tile([B, D], F32, tag="gum")
        nc.sync.dma_start(BA[:], p["blobA"].ap()[t].rearrange(
            "p (k n) -> p k n", n=A_LINE))
        nc.sync.dma_start(BA2[:], p["blobA2"].ap()[t])
        nc.scalar.dma_start(BB[:], p["blobB"].ap()[t])
        nc.scalar.dma_start(GUM[:], p["gum"].ap()[t])

        # gumbel + mask bias for this iteration (off critical path)
        nc.vector.tensor_tensor(GMEM[:], GUM[:], NEGMEM[:], ALU.add)

        # PSUM banks allocated up front so warmth dummies can recycle them
        ps_la = psum.tile([B, 512], F32, tag="la")
        ps_lb = psum.tile([B, D - 512], F32, tag="lb")

        # ---- LSTM cell (3-pass hi/lo, weights resident) ----
        # host packs gates as [i, f, o, g] so the three sigmoids are one op
        ps_g = psum.tile([H, 4, B], F32, tag="g")
        for j in range(4):
            sl = slice(j * H, (j + 1) * H)
            nc.tensor.matmul(ps_g[:, j, :], WIHHI[:, sl], AHI[:],
                             start=True, stop=False)
            nc.tensor.matmul(ps_g[:, j, :], WIHHI[:, sl], ALO[:],
                             start=False, stop=False)
            nc.tensor.matmul(ps_g[:, j, :], WIHLO[:, sl], AHI[:],
                             start=False, stop=False)
            nc.tensor.matmul(ps_g[:, j, :], WHHHI[:, sl], HHI[:],
                             start=False, stop=False)
            nc.tensor.matmul(ps_g[:, j, :], WHHHI[:, sl], HLO[:],
                             start=False, stop=False)
            nc.tensor.matmul(ps_g[:, j, :], WHHLO[:, sl], HHI[:],
                             start=False, stop=True)
        nc.scalar.activation(SIG3[:], ps_g[:, 0:3, :], ACTF.Sigmoid)
        nc.scalar.activation(TG[:], ps_g[:, 3, :], ACTF.Tanh)
        # warmth dummy: PE busy while the LSTM elementwise chain runs
        nc.tensor.matmul(ps_la[:], SIG3[:, 0, :], DUMC[:],
                         start=True, stop=True)
        nc.vector.tensor_tensor(U0[:], SIG3[:, 1, :], C32[:], ALU.mult)
        nc.vector.tensor_tensor(U1[:], SIG3[:, 0, :], TG[:], ALU.mult)
        nc.vector.tensor_tensor(C32[:], U0[:], U1[:], ALU.add)
        nc.scalar.activation(TC_[:], C32[:], ACTF.Tanh)
        nc.tensor.matmul(ps_lb[:], TC_[:], DUMC[:, 0:D - 512],
                         start=True, stop=True)
        nc.vector.tensor_tensor(H32[:], SIG3[:, 2, :], TC_[:], ALU.mult)
        nc.vector.tensor_copy(HHI[:], H32[:])
        nc.vector.tensor_tensor(HLO[:], H32[:], HHI[:], ALU.subtract)

        # ---- gating MLP: g1 = lrelu(h @ Wg1) (3-pass) ----
        ps_g1 = psum.tile([H, 2, B], F32, tag="g1")
        for m in range(2):
            whi = BB[:, B_WG1HI + m * H:B_WG1HI + (m + 1) * H]
            wlo = BB[:, B_WG1LO + m * H:B_WG1LO + (m + 1) * H]
            _mm3(nc, ps_g1[:, m, :], whi, wlo, HHI[:], HLO[:], True, True)
        nc.scalar.activation(G1F[:], ps_g1[:], ACTF.Prelu, alpha=0.2)

        # ---- logits: g1 @ Wg2, streamed true fp32 ----
        WG2 = BB[:, B_WG2:B_WG2 + 3136].bitcast(F32)   # [128, 1568]
        nc.tensor.matmul(ps_la[:], G1F[:, 0, :], WG2[:, 0:512],
                         start=True, stop=False)
        nc.tensor.matmul(ps_la[:], G1F[:, 1, :], WG2[:, 784:1296],
                         start=False, stop=True)
        nc.tensor.matmul(ps_lb[:], G1F[:, 0, :], WG2[:, 512:784],
                         start=True, stop=False)
        nc.tensor.matmul(ps_lb[:], G1F[:, 1, :], WG2[:, 1296:1568],
                         start=False, stop=True)

        # ---- sampling: argmax(pert) ----
        ps_sm = psum.tile([128, 164], F32, tag="sm")
        nc.vector.tensor_tensor(PERT[:, 0:512], ps_la[:], GMEM[:, 0:512],
                                ALU.add)
        # max over the first half runs while the second half is still adding
        nc.vector.max(MAX2[:, 0, :], PERT[:, 0:512])
        nc.vector.tensor_tensor(PERT[:, 512:D], ps_lb[:], GMEM[:, 512:D],
                                ALU.add)
        # dummy matmuls: keep the PE activity monitor busy through the
        # DVE-only sampling window so HAM holds the 2.4 GHz clock.
        # They recycle the consumed ps_la/ps_lb banks (WAR-ordered by Tile).
        nc.tensor.matmul(ps_la[:], PERT[:, 0:32], IOTA32[:, 0:512],
                         start=True, stop=True)
        nc.vector.max(MAX2[:, 1, :], PERT[:, 512:D])
        nc.tensor.matmul(ps_lb[:], PERT[:, 32:64], IOTA32[:, 0:D - 512],
                         start=True, stop=True)
        nc.vector.max(MAX8[:], MAX2[:])
        nc.tensor.matmul(ps_la[0:8, 0:512], MAX8[:], IOTA32[:, 0:512],
                         start=True, stop=True)
        nc.vector.max_index(IDX8[:], MAX8[:], PERT[:])
        nc.vector.tensor_copy(IDXF[:], IDX8[:, 0:1])
        nc.tensor.matmul(ps_lb[0:1, 0:272], IDXF[:], IOTA32[:, 0:272],
                         start=True, stop=True)
        nc.vector.tensor_scalar(HARDBF[:], IOTA32[:], IDXF[:], None,
                                ALU.is_equal)

        # one-hot -> feature-major, mask update, y = mask * x (hi/lo)
        ps_tr = psum.tile([128, NK, B], BF16, tag="tr")
        for k in range(NK):
            nc.tensor.transpose(ps_tr[0:CH[k], k, :],
                                HARDBF[:, KOF[k]:KOF[k] + CH[k]], IDENTBF[:])
        nc.vector.tensor_tensor(MEMBF[:, 0:6, :], ps_tr[:, 0:6, :],
                                MEMBF[:, 0:6, :], ALU.add)
        nc.vector.tensor_tensor(MEMBF[0:16, 6, :], ps_tr[0:16, 6, :],
                                MEMBF[0:16, 6, :], ALU.add)
        nc.vector.tensor_tensor(YTHI[:], MEMBF[:], XHI[:], ALU.mult)
        ytlo_i = nc.vector.tensor_tensor(YTLO[:], MEMBF[:], XLO[:], ALU.mult)
        # mask bookkeeping for the next iteration — explicitly ordered after
        # YTLO so the in-order DVE queue never delays the W1 block on these
        nh_i = nc.vector.tensor_scalar(NEGHARD[:], IOTA32[:], IDXF[:], NEGBIG,
                                       ALU.is_equal, ALU.mult)
        _add_dep(nh_i.ins, ytlo_i.ins, "push mask bookkeeping past YT")
        nc.vector.tensor_tensor(NEGMEM[:], NEGHARD[:], NEGMEM[:], ALU.add)

        # ---- f1 layer 1: [D -> D] ----
        ps_a1 = psum.tile([128, NK, B], F32, tag="a1")
        for m in range(NK):
            mof, mw = KOF[m], CH[m]
            for k in range(NK):
                if k < 6:
                    whi = BA[:, k, A_W1HI + mof:A_W1HI + mof + mw]
                    wlo = BA[:, k, A_W1LO + mof:A_W1LO + mof + mw]
                else:
                    whi = BA2[:, A_W1HI + mof:A_W1HI + mof + mw]
                    wlo = BA2[:, A_W1LO + mof:A_W1LO + mof + mw]
                _mm3(nc, ps_a1[0:mw, m, :], whi, wlo,
                     YTHI[0:CH[k], k, :], YTLO[0:CH[k], k, :],
                     k == 0, k == NK - 1)
        nc.scalar.activation(A1F[:], ps_a1[:], ACTF.Prelu, alpha=0.2)
        nc.tensor.matmul(ps_la[:], A1F[:, 0, :], DUMC[:],
                         start=True, stop=True)
        nc.vector.tensor_copy(A1HI[:], A1F[:])
        nc.vector.tensor_tensor(A1LO[:], A1F[:], A1HI[:], ALU.subtract)

        # ---- f1 layer 2: [D -> G] ----
        for m in range(2):
            out = ps_sm[:, m * B:(m + 1) * B]
            for k in range(NK):
                if k < 6:
                    whi = BA[:, k, A_W2HI + m * H:A_W2HI + (m + 1) * H]
                    wlo = BA[:, k, A_W2LO + m * H:A_W2LO + (m + 1) * H]
                else:
                    whi = BA2[:, A_W2HI + m * H:A_W2HI + (m + 1) * H]
                    wlo = BA2[:, A_W2LO + m * H:A_W2LO + (m + 1) * H]
                _mm3(nc, out, whi, wlo,
                     A1HI[0:CH[k], k, :], A1LO[0:CH[k], k, :],
                     k == 0, k == NK - 1)
        nc.scalar.activation(A2F[:], ps_sm[:, 0:2 * B].rearrange(
            "p (m b) -> p m b", m=2), ACTF.Prelu, alpha=0.2)
        nc.tensor.matmul(ps_lb[:], A2F[:, 0, :], DUMC[:, 0:272],
                         start=True, stop=True)
        nc.vector.tensor_copy(A2HI[:], A2F[:])
        nc.vector.tensor_tensor(A2LO[:], A2F[:], A2HI[:], ALU.subtract)

        # ---- f1 layer 3: [G -> H] ----
        for k in range(2):
            whi = BB[:, B_W3HI + k * H:B_W3HI + (k + 1) * H]
            wlo = BB[:, B_W3LO + k * H:B_W3LO + (k + 1) * H]
            _mm3(nc, ps_sm[:, 64:96], whi, wlo,
                 A2HI[:, k, :], A2LO[:, k, :], k == 0, k == 1)
        nc.scalar.activation(A3F[:], ps_sm[:, 64:96], ACTF.Prelu, alpha=0.2)
        nc.tensor.matmul(ps_la[:], A3F[:], DUMC[:],
                         start=True, stop=True)
        nc.vector.tensor_copy(A3HI[:], A3F[:])
        nc.vector.tensor_tensor(A3LO[:], A3F[:], A3HI[:], ALU.subtract)

        # ---- f1 layer 4: [H -> H] ----
        _mm3(nc, ps_sm[:, 96:128], BB[:, B_W4HI:B_W4HI + H],
             BB[:, B_W4LO:B_W4LO + H], A3HI[:], A3LO[:], True, True)
        nc.scalar.activation(A4F[:], ps_sm[:, 96:128], ACTF.Prelu, alpha=0.2)
        nc.tensor.matmul(ps_lb[:], A4F[:], DUMC[:, 0:272],
                         start=True, stop=True)
        nc.vector.tensor_copy(AHI[:], A4F[:])
        nc.vector.tensor_tensor(ALO[:], A4F[:], AHI[:], ALU.subtract)

        # ---- classifier head (bf16 single-pass; off the recurrence) ----
        nc.tensor.matmul(ps_sm[0:B, 128:138], AHI[:],
                         BB[:, B_WF2:B_WF2 + 10], start=True, stop=True)
        nc.scalar.copy(SOUT[:, :, t], ps_sm[0:B, 128:138])

    nc.sync.dma_start(p["out"].ap(), SOUT[:].rearrange("b c t -> b (c t)"))


_CACHE = {}


def _get_nc(w_bufs=2):
    key = ("nc", w_bufs)
    if key not in _CACHE:
        nc = bass.Bass("TRN2", target_bir_lowering=False, debug=False)
        p = _declare_params(nc)
        with _TileContextSplitDrain(nc) as tc:
            with ExitStack() as ctx:
                _build(ctx, tc, p, w_bufs=w_bufs)
        _split_multi_waits(nc)
        _CACHE[key] = nc
    return _CACHE[key]


def _split_np(a):
    hi = a.astype(ml_dtypes.bfloat16)
    lo = (a - hi.astype(np.float32)).astype(ml_dtypes.bfloat16)
    return hi, lo


def _prepare_in_maps(inputs):
    f = lambda k: np.ascontiguousarray(np.asarray(inputs[k]), dtype=np.float32)
    x = f("x")
    gumbel = f("gumbel")
    bg2 = f("bg2")
    gum_all = gumbel + bg2[:, None, :]          # fold bg2 into the noise
    # remaining biases are zeros in this problem; verify cheaply
    for bn in ("b1", "b2", "b3", "b4", "bf2", "bg1", "bih", "bhh"):
        if bn in inputs and np.any(np.asarray(inputs[bn])):
            raise NotImplementedError(f"nonzero bias {bn} not supported")

    W1, W2, W3, W4 = f("W1"), f("W2"), f("W3"), f("W4")
    Wf2, Wg1, Wg2 = f("Wf2"), f("Wg1"), f("Wg2")
    Wih, Whh = f("Wih"), f("Whh")
    bf = ml_dtypes.bfloat16

    # repack LSTM gates from torch order [i,f,g,o] to [i,f,o,g] so the
    # kernel applies sigmoid to the first three gate blocks in one op
    reord = lambda w: np.concatenate(
        [w[:, 0:128], w[:, 128:256], w[:, 384:512], w[:, 256:384]], axis=1)
    Wih, Whh = reord(Wih), reord(Whh)

    W1hi, W1lo = _split_np(W1)
    W2hi, W2lo = _split_np(W2)
    W3hi, W3lo = _split_np(W3)
    W4hi, W4lo = _split_np(W4)
    Wg1hi, Wg1lo = _split_np(Wg1)
    Wihhi, Wihlo = _split_np(Wih)
    Whhhi, Whhlo = _split_np(Whh)
    Wf2hi = Wf2.astype(bf)

    def chunk6(w):   # [T, 768, n] -> [T, 128, 6, n]
        n = w.shape[2]
        return np.ascontiguousarray(
            w[:, 0:768].reshape(T, 6, 128, n).transpose(0, 2, 1, 3))

    blobA = np.zeros((T, 128, 6, A_LINE), dtype=bf)
    blobA[:, :, :, A_W1HI:A_W1HI + 784] = chunk6(W1hi)
    blobA[:, :, :, A_W1LO:A_W1LO + 784] = chunk6(W1lo)
    blobA[:, :, :, A_W2HI:A_W2HI + 256] = chunk6(W2hi)
    blobA[:, :, :, A_W2LO:A_W2LO + 256] = chunk6(W2lo)
    blobA = blobA.reshape(T, 128, 6 * A_LINE)

    blobA2 = np.zeros((T, 16, A_LINE), dtype=bf)
    blobA2[:, :, A_W1HI:A_W1HI + 784] = W1hi[:, 768:784]
    blobA2[:, :, A_W1LO:A_W1LO + 784] = W1lo[:, 768:784]
    blobA2[:, :, A_W2HI:A_W2HI + 256] = W2hi[:, 768:784]
    blobA2[:, :, A_W2LO:A_W2LO + 256] = W2lo[:, 768:784]

    blobB = np.zeros((T, 128, B_LINE), dtype=bf)
    wg2pk = np.ascontiguousarray(
        Wg2.reshape(T, 2, 128, 784).transpose(0, 2, 1, 3))  # [T,128,2,784] f32
    blobB[:, :, B_WG2:B_WG2 + 3136] = wg2pk.reshape(T, 128, 1568).view(bf)
    blobB[:, :, B_WG1HI:B_WG1HI + 256] = Wg1hi
    blobB[:, :, B_WG1LO:B_WG1LO + 256] = Wg1lo
    w3pk = lambda w: w.reshape(T, 2, 128, 128).transpose(
        0, 2, 1, 3).reshape(T, 128, 256)
    blobB[:, :, B_W3HI:B_W3HI + 256] = w3pk(W3hi)
    blobB[:, :, B_W3LO:B_W3LO + 256] = w3pk(W3lo)
    blobB[:, :, B_W4HI:B_W4HI + 128] = W4hi
    blobB[:, :, B_W4LO:B_W4LO + 128] = W4lo
    blobB[:, :, B_WF2:B_WF2 + 10] = Wf2hi

    shared = {
        "blobA": blobA, "blobA2": blobA2, "blobB": blobB,
        "wihhi": Wihhi, "wihlo": Wihlo, "whhhi": Whhhi, "whhlo": Whhlo,
        "iota32": np.tile(np.arange(D, dtype=np.float32), (B, 1)),
        "identbf": np.eye(B, dtype=np.float32).astype(bf),
    }

    in_maps = []
    for c in range(NCORES):
        sl = slice(c * B, (c + 1) * B)
        m = dict(shared)
        xT = np.zeros((128, NK, B), dtype=np.float32)
        xc = x[sl].T                                  # [D, B]
        for k in range(NK):
            xT[0:CH[k], k, :] = xc[KOF[k]:KOF[k] + CH[k]]
        xthi, xtlo = _split_np(xT)
        m["xhi"] = xthi
        m["xlo"] = xtlo
        m["gum"] = np.ascontiguousarray(gum_all[:, sl])
        in_maps.append(m)
    return in_maps


def _assemble_out(res):
    return np.concatenate(
        [res.results[c]["out"].reshape(B, 10, T) for c in range(NCORES)],
        axis=0).astype(np.float32)


def kernel(**inputs) -> np.ndarray:
    in_maps = _prepare_in_maps(inputs)
    nc = _get_nc()
    res = run_bass_kernel_spmd(nc, in_maps, list(range(NCORES)))
    return _assemble_out(res)


# revision 26
# speedup vs baseline: 1.0938x; 1.0938x over previous
"""Trainium2 Bass kernel for nn_Network_81862076662591 (sampling network).

Self-contained: takes FULL inputs (as produced by the problem's
setup_inputs), data-parallel shards batch B=256 over 8 NeuronCores
(32 rows each, per-iteration weights replicated), runs the fused
LSTM + gumbel-argmax sampling + MLP scan on-device, and returns the
full [256, 10, 100] output.

Numerical scheme: every weight-stationary matmul runs as a 3-pass
bf16 hi/lo decomposition (A_hi@W_hi + A_lo@W_hi + A_hi@W_lo), which
is exact to ~2^-18 — verified offline to reproduce the fp32 argmax
trajectory exactly (min top-2 gumbel gap is 6.1e-5, pert error
~1.7e-6).  The streamed Wg2 matmul stays true fp32 (2 cyc/row).
Weights are host-packed into per-iteration partition-major blobs so
each weight DMA is 128 contiguous multi-KB lines (near line-rate).
Dummy matmuls are threaded through the sampling gap to keep the PE
HAM clock at full rate.
"""
from contextlib import ExitStack

import numpy as np
import ml_dtypes

import concourse.bass as bass
import concourse.mybir as mybir
import concourse.tile as tile
from concourse.vector_clock import ScopedClock
from concourse.bass_utils import run_bass_kernel_spmd

F32 = mybir.dt.float32
BF16 = mybir.dt.bfloat16
F16 = mybir.dt.float16
U32 = mybir.dt.uint32
ALU = mybir.AluOpType
ACTF = mybir.ActivationFunctionType

NCORES = 8
B = 32          # per-core batch
D = 784
G = 256
H = 128
T = 100
NEGBIG = -1.0e9
NK = 7                       # D chunks: 6 x 128 + 1 x 16
CH = [128] * 6 + [16]
KOF = [128 * k for k in range(7)]
# blobA free-dim offsets (per k-chunk, in bf16 elems)
A_W1HI, A_W1LO, A_W2HI, A_W2LO, A_LINE = 0, 784, 1568, 1824, 2080
# blobB free-dim offsets (bf16 elems)
B_WG2, B_WG1HI, B_WG1LO, B_W3HI, B_W3LO, B_W4HI, B_W4LO, B_WF2, B_LINE = (
    0, 3136, 3392, 3648, 3904, 4160, 4288, 4416, 4426)


class _TileContextSplitDrain(tile.TileContext):
    """This walrus build rejects >1 sem-wait on the kernel-tail Drain;
    split the accumulated waits across several sequential drains."""

    def _drain_and_barrier(self, tick_clock, wait_clock):
        drain_inst = self.nc.sync.drain()
        wait_clock.add_sem_waits(
            drain_inst.ins, ScopedClock({None: tick_clock.global_clock}))
        si = drain_inst.ins.sync_info
        waits = list(si.on_wait or []) if si is not None else []
        if len(waits) > 1:
            si.on_wait = [waits[0]]
            for w in waits[1:]:
                d2 = self.nc.sync.drain()
                if d2.ins.sync_info is None:
                    d2.ins.sync_info = mybir.SyncInfo(on_wait=[w], on_update=[])
                else:
                    d2.ins.sync_info.on_wait = [w]
        self.nc.all_engine_barrier()
        assert self.sems is not None
        popped = self.nc._tile_sem_poison_stack.pop()
        assert popped is self._sem_poison
        self.nc.clear_and_free_semaphores(list(self.sems.allocated().values()))
        self.nc.all_engine_barrier()


def _split_multi_waits(nc, limit=1):
    """This walrus accepts only `limit` sem-waits per instruction; move the
    excess onto same-engine sequencer NOPs inserted immediately before."""
    import copy

    proto = nc.vector.isa(nc.isa.Opcode.NEURON_ISA_TPB_OPCODE_NOP, {}).ins
    nop_ctr = [0]

    def make_nop(engine, waits):
        nop = copy.deepcopy(proto)
        nop_ctr[0] += 1
        nop.name = f"waitnop-{nop_ctr[0]}"
        nop.engine = engine
        nop.sync_info = mybir.SyncInfo(on_wait=list(waits), on_update=[])
        return nop

    skip = ("InstAllEngineBarrier", "InstEventSemaphore")
    for fn in nc.m.functions:
        for bb in fn.blocks:
            insts = bb.instructions
            if insts and insts[-1] is proto:
                insts.pop()
            out = []
            for inst in insts:
                si = inst.sync_info
                waits = list(si.on_wait or []) if si is not None else []
                if len(waits) > limit and type(inst).__name__ not in skip:
                    for i in range(0, len(waits) - limit, limit):
                        out.append(make_nop(inst.engine, waits[i:i + limit]))
                    si.on_wait = waits[len(waits) - limit:]
                out.append(inst)
            bb.instructions[:] = out


def _declare_params(nc):
    dp = nc.declare_dram_parameter
    p = {}
    p["blobA"] = dp("blobA", [T, 128, 6 * A_LINE], BF16, isOutput=False)
    p["blobA2"] = dp("blobA2", [T, 16, A_LINE], BF16, isOutput=False)
    p["blobB"] = dp("blobB", [T, 128, B_LINE], BF16, isOutput=False)
    p["gum"] = dp("gum", [T, B, D], F32, isOutput=False)
    p["wihhi"] = dp("wihhi", [H, 4 * H], BF16, isOutput=False)
    p["wihlo"] = dp("wihlo", [H, 4 * H], BF16, isOutput=False)
    p["whhhi"] = dp("whhhi", [H, 4 * H], BF16, isOutput=False)
    p["whhlo"] = dp("whhlo", [H, 4 * H], BF16, isOutput=False)
    p["xhi"] = dp("xhi", [128, NK, B], BF16, isOutput=False)
    p["xlo"] = dp("xlo", [128, NK, B], BF16, isOutput=False)
    p["iota32"] = dp("iota32", [B, D], F32, isOutput=False)
    p["identbf"] = dp("identbf", [B, B], BF16, isOutput=False)
    p["out"] = dp("out", [B, 10 * T], F32, isOutput=True)
    return p


def _add_dep(after_ins, before_ins, reason):
    from bass_rust import add_dep_helper
    add_dep_helper(after_ins, before_ins, reason=reason)


def _mm3(nc, out, whi, wlo, ahi, alo, first, last):
    """3-pass hi/lo matmul accumulate: out += ahi'whi + alo'whi + ahi'wlo."""
    nc.tensor.matmul(out, whi, ahi, start=first, stop=False)
    nc.tensor.matmul(out, whi, alo, start=False, stop=False)
    nc.tensor.matmul(out, wlo, ahi, start=False, stop=last)


def _build(ctx, tc, p, w_bufs=2):
    nc = tc.nc

    const_pool = ctx.enter_context(tc.tile_pool(name="const", bufs=1))
    state_pool = ctx.enter_context(tc.tile_pool(name="state", bufs=1))
    wpool = ctx.enter_context(tc.tile_pool(name="w", bufs=w_bufs))
    psum = ctx.enter_context(tc.tile_pool(name="ps", bufs=1, space="PSUM"))

    WIHHI = const_pool.tile([H, 4 * H], BF16, tag="wihhi")
    WIHLO = const_pool.tile([H, 4 * H], BF16, tag="wihlo")
    WHHHI = const_pool.tile([H, 4 * H], BF16, tag="whhhi")
    WHHLO = const_pool.tile([H, 4 * H], BF16, tag="whhlo")
    XHI = const_pool.tile([128, NK, B], BF16, tag="xhi")
    XLO = const_pool.tile([128, NK, B], BF16, tag="xlo")
    IOTA32 = const_pool.tile([B, D], F32, tag="iota32")
    IDENTBF = const_pool.tile([B, B], BF16, tag="identbf")
    DUMC = const_pool.tile([128, 512], F32, tag="dumc")   # dummy-MM fodder
    nc.vector.memset(DUMC[:], 0.0)
    for name, tl in (("wihhi", WIHHI), ("wihlo", WIHLO), ("whhhi", WHHHI),
                     ("whhlo", WHHLO), ("xhi", XHI), ("xlo", XLO),
                     ("iota32", IOTA32), ("identbf", IDENTBF)):
        nc.sync.dma_start(tl[:], p[name].ap())

    # state
    AHI = state_pool.tile([H, B], BF16, tag="ahi")    # lin hi/lo
    ALO = state_pool.tile([H, B], BF16, tag="alo")
    HHI = state_pool.tile([H, B], BF16, tag="hhi")
    HLO = state_pool.tile([H, B], BF16, tag="hlo")
    H32 = state_pool.tile([H, B], F32, tag="h32")
    C32 = state_pool.tile([H, B], F32, tag="c32")
    SIG3 = state_pool.tile([H, 3, B], F32, tag="sig3")
    TG = state_pool.tile([H, B], F32, tag="tg")
    TC_ = state_pool.tile([H, B], F32, tag="tc")
    U0 = state_pool.tile([H, B], F32, tag="u0")
    U1 = state_pool.tile([H, B], F32, tag="u1")
    G1F = state_pool.tile([H, 2, B], F32, tag="g1f")
    MEMBF = state_pool.tile([128, NK, B], BF16, tag="membf")
    YTHI = state_pool.tile([128, NK, B], BF16, tag="ythi")
    YTLO = state_pool.tile([128, NK, B], BF16, tag="ytlo")
    A1F = state_pool.tile([128, NK, B], F32, tag="a1f")
    A1HI = state_pool.tile([128, NK, B], BF16, tag="a1hi")
    A1LO = state_pool.tile([128, NK, B], BF16, tag="a1lo")
    A2F = state_pool.tile([H, 2, B], F32, tag="a2f")
    A2HI = state_pool.tile([H, 2, B], BF16, tag="a2hi")
    A2LO = state_pool.tile([H, 2, B], BF16, tag="a2lo")
    A3F = state_pool.tile([H, B], F32, tag="a3f")
    A3HI = state_pool.tile([H, B], BF16, tag="a3hi")
    A3LO = state_pool.tile([H, B], BF16, tag="a3lo")
    A4F = state_pool.tile([H, B], F32, tag="a4f")
    NEGMEM = state_pool.tile([B, D], F32, tag="negmem")
    GMEM = state_pool.tile([B, D], F32, tag="gmem")
    PERT = state_pool.tile([B, D], F32, tag="pert")
    MAX2 = state_pool.tile([B, 2, 8], F32, tag="max2")
    MAX8 = state_pool.tile([B, 8], F32, tag="max8")
    IDX8 = state_pool.tile([B, 8], U32, tag="idx8")
    IDXF = state_pool.tile([B, 1], F32, tag="idxf")
    HARDBF = state_pool.tile([B, D], BF16, tag="hardbf")
    NEGHARD = state_pool.tile([B, D], F32, tag="neghard")
    SOUT = state_pool.tile([B, 10, T], F32, tag="sout")

    nc.vector.memset(AHI[:], 0.0)
    nc.vector.memset(ALO[:], 0.0)
    nc.vector.memset(HHI[:], 0.0)
    nc.vector.memset(HLO[:], 0.0)
    nc.vector.memset(C32[:], 0.0)
    nc.vector.memset(MEMBF[:], 0.0)
    nc.vector.memset(NEGMEM[:], 0.0)

    for t in range(T):
        BA = wpool.tile([128, 6, A_LINE], BF16, tag="ba")
        BA2 = wpool.tile([16, A_LINE], BF16, tag="ba2")
        BB = wpool.tile([128, B_LINE], BF16, tag="bb")
        GUM = wpool.tile([B, D], F32, tag="gum")
        nc.sync.dma_start(BA[:], p["blobA"].ap()[t].rearrange(
            "p (k n) -> p k n", n=A_LINE))
        nc.sync.dma_start(BA2[:], p["blobA2"].ap()[t])
        nc.scalar.dma_start(BB[:], p["blobB"].ap()[t])
        nc.scalar.dma_start(GUM[:], p["gum"].ap()[t])

        # gumbel + mask bias for this iteration (off critical path)
        nc.vector.tensor_tensor(GMEM[:], GUM[:], NEGMEM[:], ALU.add)

        # PSUM banks allocated up front so warmth dummies can recycle them
        ps_la = psum.tile([B, 512], F32, tag="la")
        ps_lb = psum.tile([B, D - 512], F32, tag="lb")

        # ---- LSTM cell (3-pass hi/lo, weights resident) ----
        # host packs gates as [i, f, o, g] so the three sigmoids are one op
        ps_g = psum.tile([H, 4, B], F32, tag="g")
        for j in range(4):
            sl = slice(j * H, (j + 1) * H)
            nc.tensor.matmul(ps_g[:, j, :], WIHHI[:, sl], AHI[:],
                             start=True, stop=False)
            nc.tensor.matmul(ps_g[:, j, :], WIHHI[:, sl], ALO[:],
                             start=False, stop=False)
            nc.tensor.matmul(ps_g[:, j, :], WIHLO[:, sl], AHI[:],
                             start=False, stop=False)
            nc.tensor.matmul(ps_g[:, j, :], WHHHI[:, sl], HHI[:],
                             start=False, stop=False)
            nc.tensor.matmul(ps_g[:, j, :], WHHHI[:, sl], HLO[:],
                             start=False, stop=False)
            nc.tensor.matmul(ps_g[:, j, :], WHHLO[:, sl], HHI[:],
                             start=False, stop=True)
        nc.scalar.activation(SIG3[:], ps_g[:, 0:3, :], ACTF.Sigmoid)
        nc.scalar.activation(TG[:], ps_g[:, 3, :], ACTF.Tanh)
        nc.vector.tensor_tensor(U0[:], SIG3[:, 1, :], C32[:], ALU.mult)
        nc.vector.tensor_tensor(U1[:], SIG3[:, 0, :], TG[:], ALU.mult)
        nc.vector.tensor_tensor(C32[:], U0[:], U1[:], ALU.add)
        nc.scalar.activation(TC_[:], C32[:], ACTF.Tanh)
        nc.vector.tensor_tensor(H32[:], SIG3[:, 2, :], TC_[:], ALU.mult)
        nc.vector.tensor_copy(HHI[:], H32[:])
        nc.vector.tensor_tensor(HLO[:], H32[:], HHI[:], ALU.subtract)

        # ---- gating MLP: g1 = lrelu(h @ Wg1) (3-pass) ----
        ps_g1 = psum.tile([H, 2, B], F32, tag="g1")
        for m in range(2):
            whi = BB[:, B_WG1HI + m * H:B_WG1HI + (m + 1) * H]
            wlo = BB[:, B_WG1LO + m * H:B_WG1LO + (m + 1) * H]
            _mm3(nc, ps_g1[:, m, :], whi, wlo, HHI[:], HLO[:], True, True)
        nc.scalar.activation(G1F[:], ps_g1[:], ACTF.Prelu, alpha=0.2)

        # ---- logits: g1 @ Wg2, streamed true fp32 ----
        WG2 = BB[:, B_WG2:B_WG2 + 3136].bitcast(F32)   # [128, 1568]
        nc.tensor.matmul(ps_la[:], G1F[:, 0, :], WG2[:, 0:512],
                         start=True, stop=False)
        nc.tensor.matmul(ps_la[:], G1F[:, 1, :], WG2[:, 784:1296],
                         start=False, stop=True)
        nc.tensor.matmul(ps_lb[:], G1F[:, 0, :], WG2[:, 512:784],
                         start=True, stop=False)
        nc.tensor.matmul(ps_lb[:], G1F[:, 1, :], WG2[:, 1296:1568],
                         start=False, stop=True)

        # ---- sampling: argmax(pert) ----
        ps_sm = psum.tile([128, 164], F32, tag="sm")
        nc.vector.tensor_tensor(PERT[:, 0:512], ps_la[:], GMEM[:, 0:512],
                                ALU.add)
        # max over the first half runs while the second half is still adding
        nc.vector.max(MAX2[:, 0, :], PERT[:, 0:512])
        nc.vector.tensor_tensor(PERT[:, 512:D], ps_lb[:], GMEM[:, 512:D],
                                ALU.add)
        # dummy matmuls: keep the PE activity monitor busy through the
        # DVE-only sampling window so HAM holds the 2.4 GHz clock.
        # They recycle the consumed ps_la/ps_lb banks (WAR-ordered by Tile).
        nc.tensor.matmul(ps_la[:, 0:256], PERT[:, 0:32], PERT[:, 256:512],
                         start=True, stop=True)
        nc.tensor.matmul(ps_la[:, 256:512], PERT[:, 32:64], PERT[:, 0:256],
                         start=True, stop=True)
        nc.vector.max(MAX2[:, 1, :], PERT[:, 512:D])
        nc.tensor.matmul(ps_lb[:, 0:256], PERT[:, 64:96], PERT[:, 512:768],
                         start=True, stop=True)
        nc.vector.max(MAX8[:], MAX2[:])
        nc.tensor.matmul(ps_lb[0:8, 16:272], MAX8[:], PERT[:, 512:768],
                         start=True, stop=True)
        nc.vector.max_index(IDX8[:], MAX8[:], PERT[:])
        nc.vector.tensor_copy(IDXF[:], IDX8[:, 0:1])
        nc.tensor.matmul(ps_la[0:1, 0:256], IDXF[:], PERT[:, 256:512],
                         start=True, stop=True)
        nc.vector.tensor_scalar(HARDBF[:], IOTA32[:], IDXF[:], None,
                                ALU.is_equal)

        # one-hot -> feature-major, mask update, y = mask * x (hi/lo)
        ps_tr = psum.tile([128, NK, B], BF16, tag="tr")
        for k in range(NK):
            nc.tensor.transpose(ps_tr[0:CH[k], k, :],
                                HARDBF[:, KOF[k]:KOF[k] + CH[k]], IDENTBF[:])
        nc.vector.tensor_tensor(MEMBF[:, 0:6, :], ps_tr[:, 0:6, :],
                                MEMBF[:, 0:6, :], ALU.add)
        nc.vector.tensor_tensor(MEMBF[0:16, 6, :], ps_tr[0:16, 6, :],
                                MEMBF[0:16, 6, :], ALU.add)
        nc.vector.tensor_tensor(YTHI[:], MEMBF[:], XHI[:], ALU.mult)
        ytlo_i = nc.vector.tensor_tensor(YTLO[:], MEMBF[:], XLO[:], ALU.mult)
        # mask bookkeeping for the next iteration — explicitly ordered after
        # YTLO so the in-order DVE queue never delays the W1 block on these
        nh_i = nc.vector.tensor_scalar(NEGHARD[:], IOTA32[:], IDXF[:], NEGBIG,
                                       ALU.is_equal, ALU.mult)
        _add_dep(nh_i.ins, ytlo_i.ins, "push mask bookkeeping past YT")
        nc.vector.tensor_tensor(NEGMEM[:], NEGHARD[:], NEGMEM[:], ALU.add)

        # ---- f1 layer 1: [D -> D] ----
        ps_a1 = psum.tile([128, NK, B], F32, tag="a1")
        for m in range(NK):
            mof, mw = KOF[m], CH[m]
            for k in range(NK):
                if k < 6:
                    whi = BA[:, k, A_W1HI + mof:A_W1HI + mof + mw]
                    wlo = BA[:, k, A_W1LO + mof:A_W1LO + mof + mw]
                else:
                    whi = BA2[:, A_W1HI + mof:A_W1HI + mof + mw]
                    wlo = BA2[:, A_W1LO + mof:A_W1LO + mof + mw]
                _mm3(nc, ps_a1[0:mw, m, :], whi, wlo,
                     YTHI[0:CH[k], k, :], YTLO[0:CH[k], k, :],
                     k == 0, k == NK - 1)
        nc.scalar.activation(A1F[:], ps_a1[:], ACTF.Prelu, alpha=0.2)
        nc.vector.tensor_copy(A1HI[:], A1F[:])
        nc.vector.tensor_tensor(A1LO[:], A1F[:], A1HI[:], ALU.subtract)

        # ---- f1 layer 2: [D -> G] ----
        for m in range(2):
            out = ps_sm[:, m * B:(m + 1) * B]
            for k in range(NK):
                if k < 6:
                    whi = BA[:, k, A_W2HI + m * H:A_W2HI + (m + 1) * H]
                    wlo = BA[:, k, A_W2LO + m * H:A_W2LO + (m + 1) * H]
                else:
                    whi = BA2[:, A_W2HI + m * H:A_W2HI + (m + 1) * H]
                    wlo = BA2[:, A_W2LO + m * H:A_W2LO + (m + 1) * H]
                _mm3(nc, out, whi, wlo,
                     A1HI[0:CH[k], k, :], A1LO[0:CH[k], k, :],
                     k == 0, k == NK - 1)
        nc.scalar.activation(A2F[:], ps_sm[:, 0:2 * B].rearrange(
            "p (m b) -> p m b", m=2), ACTF.Prelu, alpha=0.2)
        nc.vector.tensor_copy(A2HI[:], A2F[:])
        nc.vector.tensor_tensor(A2LO[:], A2F[:], A2HI[:], ALU.subtract)

        # ---- f1 layer 3: [G -> H] ----
        for k in range(2):
            whi = BB[:, B_W3HI + k * H:B_W3HI + (k + 1) * H]
            wlo = BB[:, B_W3LO + k * H:B_W3LO + (k + 1) * H]
            _mm3(nc, ps_sm[:, 64:96], whi, wlo,
                 A2HI[:, k, :], A2LO[:, k, :], k == 0, k == 1)
        nc.scalar.activation(A3F[:], ps_sm[:, 64:96], ACTF.Prelu, alpha=0.2)
        nc.vector.tensor_copy(A3HI[:], A3F[:])
        nc.vector.tensor_tensor(A3LO[:], A3F[:], A3HI[:], ALU.subtract)

        # ---- f1 layer 4: [H -> H] ----
        _mm3(nc, ps_sm[:, 96:128], BB[:, B_W4HI:B_W4HI + H],
             BB[:, B_W4LO:B_W4LO + H], A3HI[:], A3LO[:], True, True)
        nc.scalar.activation(A4F[:], ps_sm[:, 96:128], ACTF.Prelu, alpha=0.2)
        nc.vector.tensor_copy(AHI[:], A4F[:])
        nc.vector.tensor_tensor(ALO[:], A4F[:], AHI[:], ALU.subtract)

        # ---- classifier head (bf16 single-pass; off the recurrence) ----
        nc.tensor.matmul(ps_sm[0:B, 128:138], AHI[:],
                         BB[:, B_WF2:B_WF2 + 10], start=True, stop=True)
        nc.scalar.copy(SOUT[:, :, t], ps_sm[0:B, 128:138])

    nc.sync.dma_start(p["out"].ap(), SOUT[:].rearrange("b c t -> b (c t)"))


_CACHE = {}


def _get_nc(w_bufs=2):
    key = ("nc", w_bufs)
    if key not in _CACHE:
        nc = bass.Bass("TRN2", target_bir_lowering=False, debug=False)
        p = _declare_params(nc)
        with _TileContextSplitDrain(nc) as tc:
            with ExitStack() as ctx:
                _build(ctx, tc, p, w_bufs=w_bufs)
        _split_multi_waits(nc)
        _CACHE[key] = nc
    return _CACHE[key]


def _split_np(a):
    hi = a.astype(ml_dtypes.bfloat16)
    lo = (a - hi.astype(np.float32)).astype(ml_dtypes.bfloat16)
    return hi, lo


def _prepare_in_maps(inputs):
    f = lambda k: np.ascontiguousarray(np.asarray(inputs[k]), dtype=np.float32)
    x = f("x")
    gumbel = f("gumbel")
    bg2 = f("bg2")
    gum_all = gumbel + bg2[:, None, :]          # fold bg2 into the noise
    # remaining biases are zeros in this problem; verify cheaply
    for bn in ("b1", "b2", "b3", "b4", "bf2", "bg1", "bih", "bhh"):
        if bn in inputs and np.any(np.asarray(inputs[bn])):
            raise NotImplementedError(f"nonzero bias {bn} not supported")

    W1, W2, W3, W4 = f("W1"), f("W2"), f("W3"), f("W4")
    Wf2, Wg1, Wg2 = f("Wf2"), f("Wg1"), f("Wg2")
    Wih, Whh = f("Wih"), f("Whh")
    bf = ml_dtypes.bfloat16

    # repack LSTM gates from torch order [i,f,g,o] to [i,f,o,g] so the
    # kernel applies sigmoid to the first three gate blocks in one op
    reord = lambda w: np.concatenate(
        [w[:, 0:128], w[:, 128:256], w[:, 384:512], w[:, 256:384]], axis=1)
    Wih, Whh = reord(Wih), reord(Whh)

    W1hi, W1lo = _split_np(W1)
    W2hi, W2lo = _split_np(W2)
    W3hi, W3lo = _split_np(W3)
    W4hi, W4lo = _split_np(W4)
    Wg1hi, Wg1lo = _split_np(Wg1)
    Wihhi, Wihlo = _split_np(Wih)
    Whhhi, Whhlo = _split_np(Whh)
    Wf2hi = Wf2.astype(bf)

    def chunk6(w):   # [T, 768, n] -> [T, 128, 6, n]
        n = w.shape[2]
        return np.ascontiguousarray(
            w[:, 0:768].reshape(T, 6, 128, n).transpose(0, 2, 1, 3))

    blobA = np.zeros((T, 128, 6, A_LINE), dtype=bf)
    blobA[:, :, :, A_W1HI:A_W1HI + 784] = chunk6(W1hi)
    blobA[:, :, :, A_W1LO:A_W1LO + 784] = chunk6(W1lo)
    blobA[:, :, :, A_W2HI:A_W2HI + 256] = chunk6(W2hi)
    blobA[:, :, :, A_W2LO:A_W2LO + 256] = chunk6(W2lo)
    blobA = blobA.reshape(T, 128, 6 * A_LINE)

    blobA2 = np.zeros((T, 16, A_LINE), dtype=bf)
    blobA2[:, :, A_W1HI:A_W1HI + 784] = W1hi[:, 768:784]
    blobA2[:, :, A_W1LO:A_W1LO + 784] = W1lo[:, 768:784]
    blobA2[:, :, A_W2HI:A_W2HI + 256] = W2hi[:, 768:784]
    blobA2[:, :, A_W2LO:A_W2LO + 256] = W2lo[:, 768:784]

    blobB = np.zeros((T, 128, B_LINE), dtype=bf)
    wg2pk = np.ascontiguousarray(
        Wg2.reshape(T, 2, 128, 784).transpose(0, 2, 1, 3))  # [T,128,2,784] f32
    blobB[:, :, B_WG2:B_WG2 + 3136] = wg2pk.reshape(T, 128, 1568).view(bf)
    blobB[:, :, B_WG1HI:B_WG1HI + 256] = Wg1hi
    blobB[:, :, B_WG1LO:B_WG1LO + 256] = Wg1lo
    w3pk = lambda w: w.reshape(T, 2, 128, 128).transpose(
        0, 2, 1, 3).reshape(T, 128, 256)
    blobB[:, :, B_W3HI:B_W3HI + 256] = w3pk(W3hi)
    blobB[:, :, B_W3LO:B_W3LO + 256] = w3pk(W3lo)
    blobB[:, :, B_W4HI:B_W4HI + 128] = W4hi
    blobB[:, :, B_W4LO:B_W4LO + 128] = W4lo
    blobB[:, :, B_WF2:B_WF2 + 10] = Wf2hi

    shared = {
        "blobA": blobA, "blobA2": blobA2, "blobB": blobB,
        "wihhi": Wihhi, "wihlo": Wihlo, "whhhi": Whhhi, "whhlo": Whhlo,
        "iota32": np.tile(np.arange(D, dtype=np.float32), (B, 1)),
        "identbf": np.eye(B, dtype=np.float32).astype(bf),
    }

    in_maps = []
    for c in range(NCORES):
        sl = slice(c * B, (c + 1) * B)
        m = dict(shared)
        xT = np.zeros((128, NK, B), dtype=np.float32)
        xc = x[sl].T                                  # [D, B]
        for k in range(NK):
            xT[0:CH[k], k, :] = xc[KOF[k]:KOF[k] + CH[k]]
        xthi, xtlo = _split_np(xT)
        m["xhi"] = xthi
        m["xlo"] = xtlo
        m["gum"] = np.ascontiguousarray(gum_all[:, sl])
        in_maps.append(m)
    return in_maps


def _assemble_out(res):
    return np.concatenate(
        [res.results[c]["out"].reshape(B, 10, T) for c in range(NCORES)],
        axis=0).astype(np.float32)


def kernel(**inputs) -> np.ndarray:
    in_maps = _prepare_in_maps(inputs)
    nc = _get_nc()
    res = run_bass_kernel_spmd(nc, in_maps, list(range(NCORES)))
    return _assemble_out(res)
